# revision 27
# baseline (speedup 1.0000x reference)
"""Bidirectional spatial Mamba block on 8 Trainium2 NeuronCores.

Sharding: core c = b*4 + dir*2 + half handles batch b, scan direction dir
(backward cores get host-reversed input; host un-reverses their output),
and d-half `half` of the DIN=192 inner channels. Each core runs an identical
SPMD program producing a [96, L] partial of out_w @ y_dir; the host sums the
four partials per batch and adds the residual x.

Device pipeline per 512-column chunk of L=16384:
  LN (channel stats via ones-matmul, broadcast back via selector-matmul)
  -> in_proj (3 matmuls) -> causal depthwise conv + SiLU (shifted-AP FMAs)
  -> x_proj (K-split matmul) -> delta = softplus (matmul + ACT)
  -> per-state-n: a = exp(A_n * delta) [ACT], b = delta*x*B_n [matmul
     broadcast + DVE mul], h = tensor_tensor_scan(a, h, b), y += h*C_n
  -> y_dir gate + out_w matmul -> DMA out.
The scan state h and the conv halo chain across chunks via SBUF tiles.
"""
import numpy as np

import concourse.bass as bass
import concourse.mybir as mybir
import concourse.tile as tile
from concourse.bass_utils import run_bass_kernel_spmd

AF = mybir.ActivationFunctionType
OP = mybir.AluOpType
FP32 = mybir.dt.float32

CH, DIN, NST, DTR, DCONV = 96, 192, 16, 6, 4
B, H, W = 2, 128, 128
L = H * W
T = 512


# ---------------------------------------------------------------- tile patch
# This walrus codegen rejects the multi-wait Drain that TileContext emits at
# exit ("Too many sync wait commands"); split the waits onto single-wait NoOps.
_PATCHED = False


def _patch_tile_drain():
    global _PATCHED
    if _PATCHED:
        return
    _PATCHED = True
    from bass_rust import ScopedClock

    def patched(self, tick_clock, wait_clock):
        nc = self.nc
        carrier = nc.sync.nop()
        wait_clock.add_sem_waits(
            carrier.ins, ScopedClock({None: tick_clock.global_clock})
        )
        si = carrier.ins.sync_info
        waits = list(si.on_wait) if si is not None else []
        if si is not None:
            si.on_wait = waits[:1]
            for w in waits[1:]:
                n2 = nc.sync.nop()
                n2.ins.sync_info = mybir.SyncInfo(on_wait=[w], on_update=[])
        nc.sync.drain()
        nc.all_engine_barrier()
        assert self.sems is not None
        popped = nc._tile_sem_poison_stack.pop()
        assert popped is self._sem_poison
        nc.clear_and_free_semaphores(list(self.sems.allocated().values()))
        nc.all_engine_barrier()

    tile.TileContext._drain_and_barrier = patched


def _split_waits(nc, max_waits=1):
    """Walrus rejects instructions carrying more than ~1 sem wait. Hoist
    extras onto same-engine NoOp carriers inserted just before."""
    for bb in nc.main_func.blocks:
        new_insts = []
        for ins in bb.instructions:
            si = ins.sync_info
            if si is not None and len(si.on_wait) > max_waits:
                waits = list(si.on_wait)
                for w in waits[max_waits:]:
                    nop = mybir.InstNoOp(
                        name=nc.get_next_instruction_name(),
                        engine=ins.engine, ins=[], outs=[],
                        sync_info=mybir.SyncInfo(on_wait=[w], on_update=[]),
                    )
                    nc.register_instruction(nop)
                    new_insts.append(nop)
                si.on_wait = waits[:max_waits]
            new_insts.append(ins)
        bb.instructions[:] = new_insts


# ---------------------------------------------------------------- builder
def build_program(nchunk=L // T):
    _patch_tile_drain()
    nc = bass.Bass(num_devices=8)
    # register the LN epsilon as a const AP so activation(bias=1e-5) lowers
    eps_t = nc.alloc_sbuf_tensor("const-f32-lneps", [128, 1], FP32)
    nc.gpsimd.memset(eps_t.ap(), 1e-5)
    nc.const_aps.aps[(FP32, 1e-5)] = eps_t.ap()
    nc.all_engine_barrier()
    Ltot = nchunk * T

    din = {}
    for name, shape in [
        ("xin", [CH, Ltot]), ("lng", [CH, 1]), ("lnb", [CH, 1]),
        ("w_in", [CH, 288]), ("cw", [CH, 8]), ("cb", [CH, 2]),
        ("w_xp", [CH, 256]), ("w_dtx", [CH, 2 * DTR]), ("w_dt", [DTR, CH]), ("dtb", [CH, 1]),
        ("asc", [CH, NST]), ("dv", [CH, 1]), ("w_out", [CH, CH]),
        ("ones1", [CH, 1]), ("ones_r", [1, CH]), ("selmat", [128, NST * CH]),
    ]:
        din[name] = nc.declare_dram_parameter(name, shape, FP32, isOutput=False)
    pout = nc.declare_dram_parameter("pout", [CH, Ltot], FP32, isOutput=True)

    with tile.TileContext(nc) as tc:
        with (
            tc.tile_pool(name="const", bufs=1) as const,
            tc.tile_pool(name="io", bufs=3) as io,
            tc.tile_pool(name="work", bufs=2) as work,
            tc.tile_pool(name="nwork", bufs=4) as nwork,
            tc.tile_pool(name="ps_misc", bufs=1, space="PSUM") as ps_misc,
            tc.tile_pool(name="ps_mm", bufs=2, space="PSUM") as ps_mm,
            tc.tile_pool(name="ps_proj", bufs=1, space="PSUM") as ps_proj,
            tc.tile_pool(name="ps_bc", bufs=3, space="PSUM") as ps_bc,
            tc.tile_pool(name="ps_out", bufs=1, space="PSUM") as ps_out,
        ):
            # persistent constants
            cst = {}
            for name in ["lng", "lnb", "w_in", "cw", "cb", "w_xp", "w_dtx", "w_dt",
                         "dtb", "asc", "dv", "w_out", "ones1", "ones_r", "selmat"]:
                t = const.tile(list(din[name].shape), FP32, tag=name)
                nc.gpsimd.dma_start(t[:], din[name].ap()[:])
                cst[name] = t
            hstate = const.tile([CH, NST], FP32, tag="hstate")
            nc.vector.memset(hstate[:], 0.0)
            halo = const.tile([CH, 6], FP32, tag="halo")
            nc.vector.memset(halo[:], 0.0)

            def _silu(src_t, dst, tg):
                # dst = src * sigmoid(src); sigmoid = exp(-ln(1+exp(-x)))
                en = work.tile([CH, T], FP32, tag=f"en_{tg}")
                nc.scalar.activation(en[:], src_t[:], AF.Exp, scale=-1.0)
                nc.scalar.activation(en[:], en[:], AF.Ln, bias=1.0)
                nc.scalar.activation(en[:], en[:], AF.Exp, scale=-1.0)
                nc.gpsimd.tensor_mul(dst[:], src_t[:], en[:])

            for ci in range(nchunk):
                sl = bass.ts(ci, T)
                # ---- Phase A: LayerNorm over channels
                xt = io.tile([CH, T], FP32, tag="xt")
                nc.gpsimd.dma_start(xt[:], din["xin"].ap()[:, sl])
                xsq = work.tile([CH, T], FP32, tag="xsq")
                nc.scalar.square(xsq[:], xt[:])
                s1 = ps_misc.tile([1, T], FP32, tag="m")
                nc.tensor.matmul(s1[:], cst["ones1"][:], xt[:])
                s2 = ps_misc.tile([1, T], FP32, tag="m")
                nc.tensor.matmul(s2[:], cst["ones1"][:], xsq[:])
                mu = work.tile([1, T], FP32, tag="mu")
                nc.scalar.mul(mu[:], s1[:], 1.0 / CH)
                ex2 = work.tile([1, T], FP32, tag="ex2")
                nc.scalar.mul(ex2[:], s2[:], 1.0 / CH)
                musq = work.tile([1, T], FP32, tag="musq")
                nc.scalar.square(musq[:], mu[:])
                var = work.tile([1, T], FP32, tag="var")
                nc.vector.tensor_sub(var[:], ex2[:], musq[:])
                lv = work.tile([1, T], FP32, tag="lv")
                nc.scalar.activation(lv[:], var[:], AF.Ln, bias=1e-5)
                rstd = work.tile([1, T], FP32, tag="rstd")
                nc.scalar.activation(rstd[:], lv[:], AF.Exp, scale=-0.5)
                mrs = work.tile([1, T], FP32, tag="mrs")
                nc.vector.tensor_mul(mrs[:], mu[:], rstd[:])
                rstd_b = ps_misc.tile([CH, T], FP32, tag="m")
                nc.tensor.matmul(rstd_b[:], cst["ones_r"][:], rstd[:])
                mrs_b = ps_misc.tile([CH, T], FP32, tag="m")
                nc.tensor.matmul(mrs_b[:], cst["ones_r"][:], mrs[:])
                t0 = work.tile([CH, T], FP32, tag="t0")
                nc.vector.tensor_mul(t0[:], xt[:], rstd_b[:])
                xn = work.tile([CH, T], FP32, tag="xn")
                nc.vector.tensor_sub(xn[:], t0[:], mrs_b[:])
                nc.vector.tensor_scalar(
                    xn[:], xn[:], cst["lng"][:], cst["lnb"][:], OP.mult, OP.add
                )
                # ---- Phase B: in_proj
                xiA = ps_mm.tile([CH, T], FP32, tag="mm")
                nc.tensor.matmul(xiA[:], cst["w_in"][:, 0:96], xn[:])
                xiB = ps_mm.tile([CH, T], FP32, tag="mm")
                nc.tensor.matmul(xiB[:], cst["w_in"][:, 96:192], xn[:])
                # ---- Phase C: conv + silu per half
                xc = []
                for hf, xi in ((0, xiA), (1, xiB)):
                    xe = work.tile([CH, T + 3], FP32, tag=f"xe{hf}")
                    nc.vector.tensor_copy(xe[:, 0:3], halo[:, 3 * hf:3 * hf + 3])
                    nc.scalar.copy(xe[:, 3:T + 3], xi[:])
                    nc.vector.tensor_copy(halo[:, 3 * hf:3 * hf + 3], xe[:, T:T + 3])
                    acc = work.tile([CH, T], FP32, tag=f"acc{hf}")
                    nc.vector.tensor_scalar(
                        acc[:], xe[:, 0:T], cst["cw"][:, 4 * hf:4 * hf + 1],
                        cst["cb"][:, hf:hf + 1], OP.mult, OP.add,
                    )
                    for j in (1, 2, 3):
                        nc.vector.scalar_tensor_tensor(
                            acc[:], xe[:, j:j + T],
                            cst["cw"][:, 4 * hf + j:4 * hf + j + 1],
                            acc[:], OP.mult, OP.add,
                        )
                    xch = work.tile([CH, T], FP32, tag=f"xc{hf}")
                    _silu(acc, xch, f"cv{hf}")
                    xc.append(xch)
                xcA, xcB = xc
                zps = ps_mm.tile([CH, T], FP32, tag="mm")
                nc.tensor.matmul(zps[:], cst["w_in"][:, 192:288], xn[:])
                zcp = work.tile([CH, T], FP32, tag="zcp")
                nc.scalar.copy(zcp[:], zps[:])
                zs = work.tile([CH, T], FP32, tag="zs")
                _silu(zcp, zs, "z")
                # ---- Phase D: x_proj
                dbl = ps_proj.tile([128, T], FP32, tag="proj")
                nc.tensor.matmul(dbl[:], cst["w_xp"][:, 0:128], xcA[:],
                                 start=True, stop=False)
                nc.tensor.matmul(dbl[:], cst["w_xp"][:, 128:256], xcB[:],
                                 start=False, stop=True)
                bcBC = work.tile([128, T], FP32, tag="bcBC")
                nc.scalar.copy(bcBC[:], dbl[:])
                dtp = ps_proj.tile([DTR, T], FP32, tag="proj")
                nc.tensor.matmul(dtp[:], cst["w_dtx"][:, 0:DTR], xcA[:],
                                 start=True, stop=False)
                nc.tensor.matmul(dtp[:], cst["w_dtx"][:, DTR:2 * DTR], xcB[:],
                                 start=False, stop=True)
                dts = work.tile([DTR, T], FP32, tag="dts")
                nc.scalar.copy(dts[:], dtp[:])
                # ---- Phase E: delta, delta*x
                dpre = ps_proj.tile([CH, T], FP32, tag="proj")
                nc.tensor.matmul(dpre[:], cst["w_dt"][:], dts[:])
                spe = work.tile([CH, T], FP32, tag="spe")
                nc.scalar.activation(spe[:], dpre[:], AF.Exp, bias=cst["dtb"][:])
                dl = work.tile([CH, T], FP32, tag="dl")
                nc.scalar.activation(dl[:], spe[:], AF.Ln, bias=1.0)
                dx = work.tile([CH, T], FP32, tag="dx")
                nc.vector.tensor_mul(dx[:], dl[:], xcA[:])
                # ---- Phase F: selective scan per state n
                yacc = work.tile([CH, T], FP32, tag="yacc")
                for n in range(NST):
                    an = nwork.tile([CH, T], FP32, tag="an")
                    nc.scalar.activation(an[:], dl[:], AF.Exp,
                                         scale=cst["asc"][:, n:n + 1])
                    Bb = ps_bc.tile([CH, T], FP32, tag="bc")
                    gB = 32 * (n % 2)
                    nc.tensor.matmul(
                        Bb[:], cst["selmat"][gB:gB + NST, CH * n:CH * n + CH],
                        bcBC[gB:gB + NST, :], tile_position=(gB, 0))
                    bn = nwork.tile([CH, T], FP32, tag="bn")
                    nc.vector.tensor_mul(bn[:], dx[:], Bb[:])
                    hn = nwork.tile([CH, T], FP32, tag="hn")
                    nc.vector.tensor_tensor_scan(
                        hn[:], an[:], bn[:], hstate[:, n:n + 1], OP.mult, OP.add
                    )
                    nc.vector.tensor_copy(hstate[:, n:n + 1], hn[:, T - 1:T])
                    Cb = ps_bc.tile([CH, T], FP32, tag="bc")
                    gC = 64 + 32 * (n % 2)
                    nc.tensor.matmul(
                        Cb[:], cst["selmat"][gC:gC + NST, CH * n:CH * n + CH],
                        bcBC[gC:gC + NST, :], tile_position=(gC, 0))
                    cbs = nwork.tile([CH, T], FP32, tag="cbs")
                    nc.scalar.copy(cbs[:], Cb[:])
                    if n == 0:
                        nc.gpsimd.tensor_mul(yacc[:], hn[:], cbs[:])
                    else:
                        yn = nwork.tile([CH, T], FP32, tag="yn")
                        nc.gpsimd.tensor_mul(yn[:], hn[:], cbs[:])
                        nc.gpsimd.tensor_add(yacc[:], yacc[:], yn[:])
                # ---- Phase G: gate + out projection
                y1 = work.tile([CH, T], FP32, tag="y1")
                nc.vector.scalar_tensor_tensor(
                    y1[:], xcA[:], cst["dv"][:], yacc[:], OP.mult, OP.add
                )
                y2 = work.tile([CH, T], FP32, tag="y2")
                nc.vector.tensor_mul(y2[:], y1[:], zs[:])
                po = ps_out.tile([CH, T], FP32, tag="out")
                nc.tensor.matmul(po[:], cst["w_out"][:], y2[:])
                pos = work.tile([CH, T], FP32, tag="pos")
                nc.scalar.copy(pos[:], po[:])
                nc.gpsimd.dma_start(pout.ap()[:, sl], pos[:])
    _split_waits(nc)
    return nc


# ---------------------------------------------------------------- host side
def _pad_xproj(xproj_w, hs, oth):
    # B rows replicated at M rows {0,32}, C at {64,96}: aligned fmap bases
    # for the 4-way tile_position broadcast matmuls
    out = np.zeros((CH, 256), np.float32)
    for ki, sel in ((0, hs), (1, oth)):
        w = xproj_w.T[sel]            # [96, 38]
        for base in (0, 32):
            out[:, ki * 128 + base:ki * 128 + base + 16] = w[:, 6:22]
        for base in (64, 96):
            out[:, ki * 128 + base:ki * 128 + base + 16] = w[:, 22:38]
    return out


def make_core_inputs(inputs, c, ltot=L):
    b, d, half = c // 4, (c // 2) % 2, c % 2
    hs = slice(half * 96, half * 96 + 96)
    oth = slice((1 - half) * 96, (1 - half) * 96 + 96)
    x = np.asarray(inputs["x"], np.float32)
    xb = x[b].reshape(CH, L)[:, :ltot]
    if d == 1:
        xb = xb[:, ::-1]
    pfx = "f_" if d == 0 else "b_"
    g = lambda n: np.asarray(inputs[pfx + n], np.float32)
    in_w = g("in_w")
    conv_w = g("conv_w")[:, 0, :]
    conv_b = g("conv_b")
    xproj_w = g("xproj_w")
    dt_w = g("dt_w")
    dt_b = g("dt_b")
    A = -np.exp(g("A_log"))
    D = g("D")
    out_w = np.asarray(inputs["out_w"], np.float32)

    selmat = np.zeros((128, NST * CH), np.float32)
    for g in range(4):
        for n in range(NST):
            selmat[32 * g + n, CH * n:CH * n + CH] = 1.0  # same pattern all groups

    c_ = np.ascontiguousarray
    return {
        "xin": c_(xb, dtype=np.float32),
        "lng": c_(np.asarray(inputs["ln_g"], np.float32)[:, None]),
        "lnb": c_(np.asarray(inputs["ln_b"], np.float32)[:, None]),
        "w_in": c_(np.concatenate(
            [in_w[hs].T, in_w[oth].T,
             in_w[192 + half * 96:192 + half * 96 + 96].T], 1)),
        "cw": c_(np.concatenate([conv_w[hs], conv_w[oth]], 1)),
        "cb": c_(np.stack([conv_b[hs], conv_b[oth]], 1)),
        "w_xp": c_(_pad_xproj(xproj_w, hs, oth)),
        "w_dtx": c_(np.concatenate([xproj_w.T[hs][:, 0:6],
                                    xproj_w.T[oth][:, 0:6]], 1)),
        "w_dt": c_(dt_w[hs].T),
        "dtb": c_(dt_b[hs][:, None]),
        "asc": c_(A[hs]),
        "dv": c_(D[hs][:, None]),
        "w_out": c_(out_w[:, hs].T),
        "ones1": np.ones((CH, 1), np.float32),
        "ones_r": np.ones((1, CH), np.float32),
        "selmat": selmat,
    }, (b, d)


_CACHE = {}


def kernel(**inputs):
    if "nc" not in _CACHE:
        _CACHE["nc"] = build_program()
    nc = _CACHE["nc"]
    in_maps, metas = [], []
    for c in range(8):
        m, meta = make_core_inputs(inputs, c)
        in_maps.append(m)
        metas.append(meta)
    res = run_bass_kernel_spmd(nc, in_maps, list(range(8)))
    x = np.asarray(inputs["x"], np.float32)
    out = x.copy()
    for c in range(8):
        b, d = metas[c]
        po = res.results[c]["pout"]
        if d == 1:
            po = po[:, ::-1]
        out[b] += po.reshape(CH, H, W)
    return out


# revision 28
# speedup vs baseline: 1.1690x; 1.1690x over previous
"""Bidirectional spatial Mamba block on 8 Trainium2 NeuronCores.

Sharding: core c = b*4 + dir*2 + half handles batch b, scan direction dir
(backward cores get host-reversed input; host un-reverses their output),
and d-half `half` of the DIN=192 inner channels. Each core runs an identical
SPMD program producing a [96, L] partial of out_w @ y_dir; the host sums the
four partials per batch and adds the residual x.

Device pipeline per 512-column chunk of L=16384:
  LN (channel stats via ones-matmul, broadcast back via selector-matmul)
  -> in_proj (3 matmuls) -> causal depthwise conv + SiLU (shifted-AP FMAs)
  -> x_proj (K-split matmul) -> delta = softplus (matmul + ACT)
  -> per-state-n: a = exp(A_n * delta) [ACT], b = delta*x*B_n [matmul
     broadcast + DVE mul], h = tensor_tensor_scan(a, h, b), y += h*C_n
  -> y_dir gate + out_w matmul -> DMA out.
The scan state h and the conv halo chain across chunks via SBUF tiles.
"""
import numpy as np

import concourse.bass as bass
import concourse.mybir as mybir
import concourse.tile as tile
from concourse.bass_utils import run_bass_kernel_spmd

AF = mybir.ActivationFunctionType
OP = mybir.AluOpType
FP32 = mybir.dt.float32

CH, DIN, NST, DTR, DCONV = 96, 192, 16, 6, 4
B, H, W = 2, 128, 128
L = H * W
T = 512


# ---------------------------------------------------------------- tile patch
# This walrus codegen rejects the multi-wait Drain that TileContext emits at
# exit ("Too many sync wait commands"); split the waits onto single-wait NoOps.
_PATCHED = False


def _patch_tile_drain():
    global _PATCHED
    if _PATCHED:
        return
    _PATCHED = True
    from bass_rust import ScopedClock

    def patched(self, tick_clock, wait_clock):
        nc = self.nc
        carrier = nc.sync.nop()
        wait_clock.add_sem_waits(
            carrier.ins, ScopedClock({None: tick_clock.global_clock})
        )
        si = carrier.ins.sync_info
        waits = list(si.on_wait) if si is not None else []
        if si is not None:
            si.on_wait = waits[:1]
            for w in waits[1:]:
                n2 = nc.sync.nop()
                n2.ins.sync_info = mybir.SyncInfo(on_wait=[w], on_update=[])
        nc.sync.drain()
        nc.all_engine_barrier()
        assert self.sems is not None
        popped = nc._tile_sem_poison_stack.pop()
        assert popped is self._sem_poison
        nc.clear_and_free_semaphores(list(self.sems.allocated().values()))
        nc.all_engine_barrier()

    tile.TileContext._drain_and_barrier = patched


def _split_waits(nc, max_waits=1):
    """Walrus rejects instructions carrying more than ~1 sem wait. Hoist
    extras onto same-engine NoOp carriers inserted just before."""
    for bb in nc.main_func.blocks:
        new_insts = []
        for ins in bb.instructions:
            si = ins.sync_info
            if si is not None and len(si.on_wait) > max_waits:
                waits = list(si.on_wait)
                for w in waits[max_waits:]:
                    nop = mybir.InstNoOp(
                        name=nc.get_next_instruction_name(),
                        engine=ins.engine, ins=[], outs=[],
                        sync_info=mybir.SyncInfo(on_wait=[w], on_update=[]),
                    )
                    nc.register_instruction(nop)
                    new_insts.append(nop)
                si.on_wait = waits[:max_waits]
            new_insts.append(ins)
        bb.instructions[:] = new_insts


# ---------------------------------------------------------------- builder
def build_program(nchunk=L // T):
    _patch_tile_drain()
    nc = bass.Bass(num_devices=8)
    # register the LN epsilon as a const AP so activation(bias=1e-5) lowers
    eps_t = nc.alloc_sbuf_tensor("const-f32-lneps", [128, 1], FP32)
    nc.gpsimd.memset(eps_t.ap(), 1e-5)
    nc.const_aps.aps[(FP32, 1e-5)] = eps_t.ap()
    nc.all_engine_barrier()
    Ltot = nchunk * T

    din = {}
    for name, shape in [
        ("xin", [CH, Ltot]), ("lng", [CH, 1]), ("lnb", [CH, 1]),
        ("w_in", [CH, 288]), ("cw", [CH, 8]), ("cb", [CH, 2]),
        ("w_xp", [CH, 256]), ("w_dtx", [CH, 2 * DTR]), ("w_dt", [DTR, CH]), ("dtb", [CH, 1]),
        ("asc", [CH, NST]), ("dv", [CH, 1]), ("w_out", [CH, CH]),
        ("ones1", [CH, 1]), ("ones_r", [1, CH]), ("selmat", [128, NST * CH]),
    ]:
        din[name] = nc.declare_dram_parameter(name, shape, FP32, isOutput=False)
    pout = nc.declare_dram_parameter("pout", [CH, Ltot], FP32, isOutput=True)

    with tile.TileContext(nc) as tc:
        with (
            tc.tile_pool(name="const", bufs=1) as const,
            tc.tile_pool(name="io", bufs=3) as io,
            tc.tile_pool(name="work", bufs=2) as work,
            tc.tile_pool(name="nwork", bufs=4) as nwork,
            tc.tile_pool(name="ps_misc", bufs=1, space="PSUM") as ps_misc,
            tc.tile_pool(name="ps_mm", bufs=2, space="PSUM") as ps_mm,
            tc.tile_pool(name="ps_proj", bufs=2, space="PSUM") as ps_proj,
            tc.tile_pool(name="ps_bc", bufs=2, space="PSUM") as ps_bc,
            tc.tile_pool(name="ps_out", bufs=1, space="PSUM") as ps_out,
        ):
            # persistent constants
            cst = {}
            for name in ["lng", "lnb", "w_in", "cw", "cb", "w_xp", "w_dtx", "w_dt",
                         "dtb", "asc", "dv", "w_out", "ones1", "ones_r", "selmat"]:
                t = const.tile(list(din[name].shape), FP32, tag=name)
                nc.gpsimd.dma_start(t[:], din[name].ap()[:])
                cst[name] = t
            hstate = const.tile([CH, NST], FP32, tag="hstate")
            nc.vector.memset(hstate[:], 0.0)
            halo = const.tile([CH, 6], FP32, tag="halo")
            nc.vector.memset(halo[:], 0.0)

            def _silu(src_t, dst, tg):
                # dst = src * sigmoid(src); sigmoid = exp(-ln(1+exp(-x)))
                en = work.tile([CH, T], FP32, tag=f"en_{tg}")
                nc.scalar.activation(en[:], src_t[:], AF.Exp, scale=-1.0)
                nc.scalar.activation(en[:], en[:], AF.Ln, bias=1.0)
                nc.scalar.activation(en[:], en[:], AF.Exp, scale=-1.0)
                nc.gpsimd.tensor_mul(dst[:], src_t[:], en[:])

            for ci in range(nchunk):
                sl = bass.ts(ci, T)
                # ---- Phase A: LayerNorm over channels
                xt = io.tile([CH, T], FP32, tag="xt")
                nc.gpsimd.dma_start(xt[:], din["xin"].ap()[:, sl])
                xsq = work.tile([CH, T], FP32, tag="xsq")
                nc.scalar.square(xsq[:], xt[:])
                s1 = ps_misc.tile([1, T], FP32, tag="m")
                nc.tensor.matmul(s1[:], cst["ones1"][:], xt[:])
                s2 = ps_misc.tile([1, T], FP32, tag="m")
                nc.tensor.matmul(s2[:], cst["ones1"][:], xsq[:])
                mu = work.tile([1, T], FP32, tag="mu")
                nc.scalar.mul(mu[:], s1[:], 1.0 / CH)
                ex2 = work.tile([1, T], FP32, tag="ex2")
                nc.scalar.mul(ex2[:], s2[:], 1.0 / CH)
                musq = work.tile([1, T], FP32, tag="musq")
                nc.scalar.square(musq[:], mu[:])
                var = work.tile([1, T], FP32, tag="var")
                nc.vector.tensor_sub(var[:], ex2[:], musq[:])
                lv = work.tile([1, T], FP32, tag="lv")
                nc.scalar.activation(lv[:], var[:], AF.Ln, bias=1e-5)
                rstd = work.tile([1, T], FP32, tag="rstd")
                nc.scalar.activation(rstd[:], lv[:], AF.Exp, scale=-0.5)
                mrs = work.tile([1, T], FP32, tag="mrs")
                nc.vector.tensor_mul(mrs[:], mu[:], rstd[:])
                rstd_b = ps_misc.tile([CH, T], FP32, tag="m")
                nc.tensor.matmul(rstd_b[:], cst["ones_r"][:], rstd[:])
                mrs_b = ps_misc.tile([CH, T], FP32, tag="m")
                nc.tensor.matmul(mrs_b[:], cst["ones_r"][:], mrs[:])
                t0 = work.tile([CH, T], FP32, tag="t0")
                nc.vector.tensor_mul(t0[:], xt[:], rstd_b[:])
                xn = work.tile([CH, T], FP32, tag="xn")
                nc.vector.tensor_sub(xn[:], t0[:], mrs_b[:])
                nc.vector.tensor_scalar(
                    xn[:], xn[:], cst["lng"][:], cst["lnb"][:], OP.mult, OP.add
                )
                # ---- Phase B: in_proj
                xiA = ps_mm.tile([CH, T], FP32, tag="mm")
                nc.tensor.matmul(xiA[:], cst["w_in"][:, 0:96], xn[:])
                xiB = ps_mm.tile([CH, T], FP32, tag="mm")
                nc.tensor.matmul(xiB[:], cst["w_in"][:, 96:192], xn[:])
                # ---- Phase C: conv + silu per half
                xc = []
                for hf, xi in ((0, xiA), (1, xiB)):
                    xe = work.tile([CH, T + 3], FP32, tag=f"xe{hf}")
                    nc.vector.tensor_copy(xe[:, 0:3], halo[:, 3 * hf:3 * hf + 3])
                    nc.scalar.copy(xe[:, 3:T + 3], xi[:])
                    nc.vector.tensor_copy(halo[:, 3 * hf:3 * hf + 3], xe[:, T:T + 3])
                    acc = work.tile([CH, T], FP32, tag=f"acc{hf}")
                    nc.vector.tensor_scalar(
                        acc[:], xe[:, 0:T], cst["cw"][:, 4 * hf:4 * hf + 1],
                        cst["cb"][:, hf:hf + 1], OP.mult, OP.add,
                    )
                    for j in (1, 2, 3):
                        nc.vector.scalar_tensor_tensor(
                            acc[:], xe[:, j:j + T],
                            cst["cw"][:, 4 * hf + j:4 * hf + j + 1],
                            acc[:], OP.mult, OP.add,
                        )
                    xch = work.tile([CH, T], FP32, tag=f"xc{hf}")
                    _silu(acc, xch, f"cv{hf}")
                    xc.append(xch)
                xcA, xcB = xc
                zps = ps_mm.tile([CH, T], FP32, tag="mm")
                nc.tensor.matmul(zps[:], cst["w_in"][:, 192:288], xn[:])
                zcp = work.tile([CH, T], FP32, tag="zcp")
                nc.scalar.copy(zcp[:], zps[:])
                zs = work.tile([CH, T], FP32, tag="zs")
                _silu(zcp, zs, "z")
                # ---- Phase D: x_proj
                dbl = ps_proj.tile([128, T], FP32, tag="proj")
                nc.tensor.matmul(dbl[:], cst["w_xp"][:, 0:128], xcA[:],
                                 start=True, stop=False)
                nc.tensor.matmul(dbl[:], cst["w_xp"][:, 128:256], xcB[:],
                                 start=False, stop=True)
                bcBC = work.tile([128, T], FP32, tag="bcBC")
                nc.scalar.copy(bcBC[:], dbl[:])
                dtp = ps_proj.tile([DTR, T], FP32, tag="proj")
                nc.tensor.matmul(dtp[:], cst["w_dtx"][:, 0:DTR], xcA[:],
                                 start=True, stop=False)
                nc.tensor.matmul(dtp[:], cst["w_dtx"][:, DTR:2 * DTR], xcB[:],
                                 start=False, stop=True)
                dts = work.tile([DTR, T], FP32, tag="dts")
                nc.scalar.copy(dts[:], dtp[:])
                # ---- Phase E: delta, delta*x
                dpre = ps_proj.tile([CH, T], FP32, tag="proj")
                nc.tensor.matmul(dpre[:], cst["w_dt"][:], dts[:])
                spe = work.tile([CH, T], FP32, tag="spe")
                nc.scalar.activation(spe[:], dpre[:], AF.Exp, bias=cst["dtb"][:])
                dl = work.tile([CH, T], FP32, tag="dl")
                nc.scalar.activation(dl[:], spe[:], AF.Ln, bias=1.0)
                dx = work.tile([CH, T], FP32, tag="dx")
                nc.vector.tensor_mul(dx[:], dl[:], xcA[:])
                # ---- Phase F: selective scan per state n
                yacc = work.tile([CH, T], FP32, tag="yacc")
                for n in range(NST):
                    an = nwork.tile([CH, T], FP32, tag="an")
                    nc.scalar.activation(an[:], dl[:], AF.Exp,
                                         scale=cst["asc"][:, n:n + 1])
                    Bb = ps_bc.tile([CH, T], FP32, tag="bc")
                    gB = 32 * (n % 2)
                    nc.tensor.matmul(
                        Bb[:], cst["selmat"][gB:gB + NST, CH * n:CH * n + CH],
                        bcBC[gB:gB + NST, :], tile_position=(gB, 0))
                    bn = nwork.tile([CH, T], FP32, tag="bn")
                    nc.vector.tensor_mul(bn[:], dx[:], Bb[:])
                    hn = nwork.tile([CH, T], FP32, tag="hn")
                    nc.vector.tensor_tensor_scan(
                        hn[:], an[:], bn[:], hstate[:, n:n + 1], OP.mult, OP.add
                    )
                    nc.vector.tensor_copy(hstate[:, n:n + 1], hn[:, T - 1:T])
                    Cb = ps_bc.tile([CH, T], FP32, tag="bc")
                    gC = 64 + 32 * (n % 2)
                    nc.tensor.matmul(
                        Cb[:], cst["selmat"][gC:gC + NST, CH * n:CH * n + CH],
                        bcBC[gC:gC + NST, :], tile_position=(gC, 0))
                    cbs = nwork.tile([CH, T], FP32, tag="cbs")
                    nc.scalar.copy(cbs[:], Cb[:])
                    if n == 0:
                        nc.gpsimd.tensor_mul(yacc[:], hn[:], cbs[:])
                    else:
                        yn = nwork.tile([CH, T], FP32, tag="yn")
                        nc.gpsimd.tensor_mul(yn[:], hn[:], cbs[:])
                        nc.gpsimd.tensor_add(yacc[:], yacc[:], yn[:])
                # ---- Phase G: gate + out projection
                y1 = work.tile([CH, T], FP32, tag="y1")
                nc.vector.scalar_tensor_tensor(
                    y1[:], xcA[:], cst["dv"][:], yacc[:], OP.mult, OP.add
                )
                y2 = work.tile([CH, T], FP32, tag="y2")
                nc.vector.tensor_mul(y2[:], y1[:], zs[:])
                po = ps_out.tile([CH, T], FP32, tag="out")
                nc.tensor.matmul(po[:], cst["w_out"][:], y2[:])
                pos = work.tile([CH, T], FP32, tag="pos")
                nc.scalar.copy(pos[:], po[:])
                nc.gpsimd.dma_start(pout.ap()[:, sl], pos[:])
    _split_waits(nc)
    return nc


# ---------------------------------------------------------------- host side
def _pad_xproj(xproj_w, hs, oth):
    # B rows replicated at M rows {0,32}, C at {64,96}: aligned fmap bases
    # for the 4-way tile_position broadcast matmuls
    out = np.zeros((CH, 256), np.float32)
    for ki, sel in ((0, hs), (1, oth)):
        w = xproj_w.T[sel]            # [96, 38]
        for base in (0, 32):
            out[:, ki * 128 + base:ki * 128 + base + 16] = w[:, 6:22]
        for base in (64, 96):
            out[:, ki * 128 + base:ki * 128 + base + 16] = w[:, 22:38]
    return out


def make_core_inputs(inputs, c, ltot=L):
    b, d, half = c // 4, (c // 2) % 2, c % 2
    hs = slice(half * 96, half * 96 + 96)
    oth = slice((1 - half) * 96, (1 - half) * 96 + 96)
    x = np.asarray(inputs["x"], np.float32)
    xb = x[b].reshape(CH, L)[:, :ltot]
    if d == 1:
        xb = xb[:, ::-1]
    pfx = "f_" if d == 0 else "b_"
    g = lambda n: np.asarray(inputs[pfx + n], np.float32)
    in_w = g("in_w")
    conv_w = g("conv_w")[:, 0, :]
    conv_b = g("conv_b")
    xproj_w = g("xproj_w")
    dt_w = g("dt_w")
    dt_b = g("dt_b")
    A = -np.exp(g("A_log"))
    D = g("D")
    out_w = np.asarray(inputs["out_w"], np.float32)

    selmat = np.zeros((128, NST * CH), np.float32)
    for g in range(4):
        for n in range(NST):
            selmat[32 * g + n, CH * n:CH * n + CH] = 1.0  # same pattern all groups

    c_ = np.ascontiguousarray
    return {
        "xin": c_(xb, dtype=np.float32),
        "lng": c_(np.asarray(inputs["ln_g"], np.float32)[:, None]),
        "lnb": c_(np.asarray(inputs["ln_b"], np.float32)[:, None]),
        "w_in": c_(np.concatenate(
            [in_w[hs].T, in_w[oth].T,
             in_w[192 + half * 96:192 + half * 96 + 96].T], 1)),
        "cw": c_(np.concatenate([conv_w[hs], conv_w[oth]], 1)),
        "cb": c_(np.stack([conv_b[hs], conv_b[oth]], 1)),
        "w_xp": c_(_pad_xproj(xproj_w, hs, oth)),
        "w_dtx": c_(np.concatenate([xproj_w.T[hs][:, 0:6],
                                    xproj_w.T[oth][:, 0:6]], 1)),
        "w_dt": c_(dt_w[hs].T),
        "dtb": c_(dt_b[hs][:, None]),
        "asc": c_(A[hs]),
        "dv": c_(D[hs][:, None]),
        "w_out": c_(out_w[:, hs].T),
        "ones1": np.ones((CH, 1), np.float32),
        "ones_r": np.ones((1, CH), np.float32),
        "selmat": selmat,
    }, (b, d)


_CACHE = {}


def kernel(**inputs):
    if "nc" not in _CACHE:
        _CACHE["nc"] = build_program()
    nc = _CACHE["nc"]
    in_maps, metas = [], []
    for c in range(8):
        m, meta = make_core_inputs(inputs, c)
        in_maps.append(m)
        metas.append(meta)
    res = run_bass_kernel_spmd(nc, in_maps, list(range(8)))
    x = np.asarray(inputs["x"], np.float32)
    out = x.copy()
    for c in range(8):
        b, d = metas[c]
        po = res.results[c]["pout"]
        if d == 1:
            po = po[:, ::-1]
        out[b] += po.reshape(CH, H, W)
    return out
